# revision 1
# baseline (speedup 1.0000x reference)
"""DynaLoRALinear Trainium2 kernel.

Data-parallel over batch B across 8 NeuronCores (one sample per core).
Per core:
  - router:  logits = pooled @ (W_r @ gating_W).T  computed as a sharded
    partial (each core contracts over a 512-wide slice of D) + AllReduce.
  - gate weights from expert_scores ranks + module_prob>0.5 branch select.
  - base:    out = x_b @ W_base.T + b_base   (tf32 matmuls, fp32 PSUM accum)
  - lora:    t = x_b @ A_cat.T (fused into chunk-0 k-loop), then
             out += t @ (B_cat * gate).T
Matmuls use float32r (tf32) operands pre-rounded on host: 1 cyc/row on PE
(4x faster than fp32) at ~3e-4 scale-relative absmax error.
"""

import sys
import types

import numpy as np

B, L, D, E, R, NMOD = 8, 2048, 4096, 4, 8, 7
N_CORES = 8
DSH = D // N_CORES  # 512: per-core slice of D for the router shard
ER = E * R          # 32
O_C = 1024          # W_base column chunk cached in SBUF
N_CHUNK = D // O_C  # 4
KT = D // 128       # 32 k-tiles
XB = 8              # k-tiles batched per x DMA
MT = L // 128       # 16 m-tiles


def _round_tf32(a) -> np.ndarray:
    """Round-to-nearest-even fp32 -> tf32 (10-bit mantissa), keep fp32 bits."""
    a = np.ascontiguousarray(a, dtype=np.float32)
    u = a.view(np.uint32).astype(np.uint64)
    u = (u + 0xFFF + ((u >> 13) & 1)) & 0xFFFFE000
    return np.ascontiguousarray(u.astype(np.uint32)).view(np.float32)


def _install_profile_hook():
    """Make bass_utils' trace path importable (no-op if already present)."""
    try:
        import antenv.axon_hooks  # noqa: F401
        return
    except ImportError:
        pass
    try:
        import antenv
    except ImportError:
        return
    mod = types.ModuleType("antenv.axon_hooks")
    mod._hook = None
    mod.set_axon_ntff_profile_hook = lambda h: setattr(mod, "_hook", h)
    mod.get_axon_ntff_profile_hook = lambda: mod._hook
    sys.modules["antenv.axon_hooks"] = mod
    antenv.axon_hooks = mod
    try:
        from trn_agent_boot.trn_boot import _ntff_profile_via_ctypes
        hook = _ntff_profile_via_ctypes("/opt/axon/libaxon_pjrt.so")
        if hook is not None:
            mod.set_axon_ntff_profile_hook(hook)
    except Exception:
        pass


_PROGRAM_CACHE = {}


def _build_program(k: int, module_idx: int, has_bias: bool):
    import concourse.mybir as mybir
    import concourse.tile as tile
    from concourse import bacc
    from concourse.masks import make_identity

    f32 = mybir.dt.float32
    f32r = mybir.dt.float32r
    alu = mybir.AluOpType
    act_fn = mybir.ActivationFunctionType

    k_lo = max(1, k // 2)

    nc = bacc.Bacc("TRN2", target_bir_lowering=False, debug=False,
                   num_devices=N_CORES)

    # --- DRAM I/O -------------------------------------------------------
    xT = nc.dram_tensor("xT", [D, L], f32r, kind="ExternalInput")
    WbT = nc.dram_tensor("WbT", [D, D], f32r, kind="ExternalInput")
    gw = nc.dram_tensor("gw", [D, DSH], f32r, kind="ExternalInput")
    WrT = nc.dram_tensor("WrT", [D, NMOD], f32r, kind="ExternalInput")
    pooledT = nc.dram_tensor("pooledT", [DSH, B], f32, kind="ExternalInput")
    scores_f = nc.dram_tensor("scores_f", [1, E * B], f32,
                              kind="ExternalInput")
    A_rhs = nc.dram_tensor("A_rhs", [D, ER], f32r, kind="ExternalInput")
    B_cat = nc.dram_tensor("B_cat", [ER, D], f32, kind="ExternalInput")
    b_row = nc.dram_tensor("b_row", [1, D], f32, kind="ExternalInput")
    msel = nc.dram_tensor("msel", [ER, E * B], f32, kind="ExternalInput")
    out = nc.dram_tensor("out", [L, D], f32, kind="ExternalOutput")

    with tile.TileContext(nc) as tc:
        with (
            tc.tile_pool(name="const", bufs=1) as const_pool,
            tc.tile_pool(name="gatep", bufs=1) as gate_pool,
            tc.tile_pool(name="rsb", bufs=1) as rsb,
            tc.tile_pool(name="wpool",
                         bufs=2 * KT + (4 if has_bias else 4)) as wpool,
            tc.tile_pool(name="xpool", bufs=10) as xpool,
            tc.tile_pool(name="apool", bufs=1) as apool,
            tc.tile_pool(name="tpool", bufs=4) as tpool,
            tc.tile_pool(name="bpool", bufs=2) as bpool,
            tc.tile_pool(name="biasp", bufs=D // 512) as biasp,
            tc.tile_pool(name="epool", bufs=4) as epool,
        ):
            ident = const_pool.tile([128, 128], f32)
            make_identity(nc, ident)
            gate32 = gate_pool.tile([ER, 1], f32)

            bias_all = []
            if has_bias:
                for hh in range(D // 512):
                    bias_bc = biasp.tile([128, 512], f32, tag="biasbc",
                                         name=f"biasbc_{hh}")
                    nc.sync.dma_start(
                        bias_bc[0:1, :],
                        b_row[:, hh * 512:(hh + 1) * 512])
                    nc.gpsimd.partition_broadcast(bias_bc[:],
                                                  bias_bc[0:1, :])
                    bias_all.append(bias_bc)

            # ====== router part A: matmuls + AllReduce trigger =========
            # (everything that depends on the AllReduce result is emitted
            # AFTER chunk 0, so the collective never blocks the in-order
            # engine queues ahead of chunk-0 work.)
            wtiles0 = []
            with (
                tc.tile_pool(name="rgw", bufs=6) as rgw,
                tc.tile_pool(name="rps", bufs=1, space="PSUM") as rps,
                tc.tile_pool(name="rdram", bufs=1, space="DRAM") as rdram,
            ):
                wr_sb = rsb.tile([128, KT, NMOD], f32r)
                nc.sync.dma_start(
                    wr_sb[:], WrT[:].rearrange("(a p) m -> p a m", p=128))
                wc_ps = rps.tile([NMOD, DSH], f32)
                for kt in range(KT):
                    gwt = rgw.tile([128, DSH], f32r, tag="gwt",
                                   name=f"gwt_{kt}")
                    nc.sync.dma_start(gwt[:], gw[kt * 128:(kt + 1) * 128, :])
                    nc.tensor.matmul(wc_ps[:], wr_sb[:, kt, :], gwt[:],
                                     start=(kt == 0), stop=(kt == KT - 1))
                # small inputs, then A, then W chunk 0 stream
                pt_sb = rsb.tile([128, 4, B], f32)
                nc.sync.dma_start(
                    pt_sb[:],
                    pooledT[:].rearrange("(a p) m -> p a m", p=128))
                msel_sb = rsb.tile([ER, E * B], f32)
                nc.sync.dma_start(msel_sb[:], msel[:])
                sc = rsb.tile([1, E * B], f32)
                nc.sync.dma_start(sc[:], scores_f[:])
                a_sb = apool.tile([128, KT, ER], f32r)
                nc.sync.dma_start(
                    a_sb[:], A_rhs[:].rearrange("(a p) m -> p a m", p=128))
                for kt in range(KT):
                    wt = wpool.tile([128, 512], f32r, tag="w",
                                    name=f"w_0_{kt}")
                    nc.sync.dma_start(wt[:],
                                      WbT[kt * 128:(kt + 1) * 128, 0:512])
                    wtiles0.append(wt)

                wc_sb = rsb.tile([NMOD, DSH], f32)
                nc.vector.tensor_copy(wc_sb[:], wc_ps[:])
                wct = rsb.tile([128, 4 * NMOD], f32)
                for j in range(4):
                    tp = rps.tile([128, NMOD], f32, tag="tp", name=f"tp_{j}")
                    nc.tensor.transpose(
                        tp[:], wc_sb[:, j * 128:(j + 1) * 128],
                        ident[0:NMOD, 0:NMOD])
                    nc.vector.tensor_copy(
                        wct[:, j * NMOD:(j + 1) * NMOD], tp[:])

                lg_ps = rps.tile([NMOD, B], f32)
                for j in range(4):
                    nc.tensor.matmul(lg_ps[:],
                                     wct[:, j * NMOD:(j + 1) * NMOD],
                                     pt_sb[:, j, :],
                                     start=(j == 0), stop=(j == 3))
                lp_sb = rsb.tile([NMOD, B], f32)
                nc.vector.tensor_copy(lp_sb[:], lg_ps[:])

                cc_in = rdram.tile([NMOD, B], f32)
                cc_out = rdram.tile([NMOD, B], f32)
                nc.gpsimd.dma_start(cc_in[:], lp_sb[:])
                nc.gpsimd.collective_compute(
                    "AllReduce", alu.add,
                    replica_groups=[list(range(N_CORES))],
                    ins=[cc_in.opt()], outs=[cc_out.opt()])
                lg_sb = rsb.tile([NMOD, B], f32)
                nc.gpsimd.dma_start(lg_sb[:], cc_out[:])

                # collective-independent: expert ranks from scores
                rank = rsb.tile([1, E * B], f32)
                nc.vector.memset(rank[:], 0.0)
                tmp = rsb.tile([1, B], f32)
                for e in range(E):
                    re = rank[:, e * B:(e + 1) * B]
                    se = sc[:, e * B:(e + 1) * B]
                    for e2 in range(E):
                        if e2 == e:
                            continue
                        s2 = sc[:, e2 * B:(e2 + 1) * B]
                        nc.vector.tensor_tensor(tmp[:], s2, se, op=alu.is_gt)
                        nc.vector.tensor_add(re, re, tmp[:])
                        if e2 < e:
                            nc.vector.tensor_tensor(tmp[:], s2, se,
                                                    op=alu.is_equal)
                            nc.vector.tensor_add(re, re, tmp[:])
                w_hi = rsb.tile([1, E * B], f32)
                nc.vector.tensor_scalar(w_hi[:], rank[:], float(k),
                                        1.0 / float(k),
                                        op0=alu.is_lt, op1=alu.mult)
                w_lo = rsb.tile([1, E * B], f32)
                nc.vector.tensor_scalar(w_lo[:], rank[:], float(k_lo),
                                        1.0 / float(k_lo),
                                        op0=alu.is_lt, op1=alu.mult)
                diff = rsb.tile([1, E * B], f32)
                nc.vector.tensor_sub(diff[:], w_hi[:], w_lo[:])

            # ====== router part B (emitted after chunk 0 below) ========
            def emit_part_b(mps):
                ltp = mps.tile([B, NMOD], f32, tag="ps", name="ltp")
                nc.tensor.transpose(ltp[:], lg_sb[:], ident[0:NMOD, 0:NMOD])
                lt = rsb.tile([B, NMOD], f32)
                nc.vector.tensor_copy(lt[:], ltp[:])
                mx = rsb.tile([B, 1], f32)
                nc.vector.tensor_reduce(out=mx[:], in_=lt[:], op=alu.max,
                                        axis=mybir.AxisListType.X)
                mxn = rsb.tile([B, 1], f32)
                nc.vector.tensor_scalar_mul(mxn[:], mx[:], -1.0)
                ex = rsb.tile([B, NMOD], f32)
                nc.scalar.activation(ex[:], lt[:], act_fn.Exp, bias=mxn[:])
                sm = rsb.tile([B, 1], f32)
                nc.vector.tensor_reduce(out=sm[:], in_=ex[:], op=alu.add,
                                        axis=mybir.AxisListType.X)
                rs = rsb.tile([B, 1], f32)
                nc.vector.reciprocal(rs[:], sm[:])
                p0 = rsb.tile([B, 1], f32)
                nc.vector.tensor_mul(
                    p0[:], ex[:, module_idx:module_idx + 1], rs[:])
                hi = rsb.tile([B, 1], f32)
                nc.vector.tensor_single_scalar(hi[:], p0[:], 0.5, alu.is_gt)
                hp = mps.tile([1, B], f32, tag="ps", name="hp")
                nc.tensor.transpose(hp[:], hi[:], ident[0:B, 0:B])
                hi_row = rsb.tile([1, B], f32)
                nc.vector.tensor_copy(hi_row[:], hp[:])
                gate = rsb.tile([1, E * B], f32)
                for e in range(E):
                    nc.vector.tensor_mul(gate[:, e * B:(e + 1) * B],
                                         diff[:, e * B:(e + 1) * B],
                                         hi_row[:])
                nc.vector.tensor_add(gate[:], gate[:], w_lo[:])
                gateb = rsb.tile([ER, E * B], f32)
                nc.gpsimd.partition_broadcast(gateb[:], gate[:])
                g32m = rsb.tile([ER, E * B], f32)
                nc.vector.tensor_tensor(g32m[:], gateb[:], msel_sb[:],
                                        op=alu.mult)
                nc.vector.tensor_reduce(out=gate32[:], in_=g32m[:],
                                        op=alu.add,
                                        axis=mybir.AxisListType.X)

            # ============== main: base + lora ==========================
            with (
                tc.tile_pool(name="mps", bufs=8, space="PSUM") as mps,
            ):
                CHUNKS = [(0, 512), (512, 1024), (1536, 1024),
                          (2560, 1024), (3584, 512)]
                tT_tiles = [None] * (MT // 4)
                for c, (col0, width) in enumerate(CHUNKS):
                    nh = width // 512
                    GS = 4
                    NG = MT // GS
                    if c == 0:
                        wtiles = [[wtiles0[kt]] for kt in range(KT)]
                    else:
                        wtiles = []
                        for kt in range(KT):
                            row = []
                            for h in range(nh):
                                wt = wpool.tile([128, 512], f32r, tag="w",
                                                name=f"w_{c}_{kt}_{h}")
                                nc.sync.dma_start(
                                    wt[:],
                                    WbT[kt * 128:(kt + 1) * 128,
                                        col0 + h * 512:col0 + (h + 1) * 512])
                                row.append(wt)
                            wtiles.append(row)
                    # B chunk halves scaled by this core's gate.
                    # Chunk 0's scale must wait for part B (emitted in the
                    # c==0 tail below), so only stage its DMA here.
                    bh = []
                    bstg0 = []
                    for h in range(nh):
                        b_stg = bpool.tile([ER, 512], f32, tag="bstg",
                                           name=f"bstg_{c}_{h}")
                        nc.sync.dma_start(
                            b_stg[:],
                            B_cat[:, col0 + h * 512:col0 + (h + 1) * 512])
                        if c == 0:
                            bstg0.append(b_stg)
                            bh.append(None)
                            continue
                        b_scl = bpool.tile([ER, 512], f32r, tag="bscl",
                                           name=f"bscl_{c}_{h}")
                        nc.vector.tensor_scalar_mul(b_scl[:], b_stg[:],
                                                    gate32[:, 0:1])
                        bh.append(b_scl)

                    for mg in range(NG):
                        pss = []
                        for mi in range(GS):
                            row = []
                            for h in range(nh):
                                ps = mps.tile([128, 512], f32, tag="ps",
                                              name=f"ps_{c}_{mg}_{mi}_{h}")
                                row.append(ps)
                            pss.append(row)
                        if c == 0:
                            ps_t = mps.tile([ER, 512], f32, tag="ps",
                                            name=f"pst_{mg}")
                        for kt in range(KT):
                            xs = xpool.tile([128, GS * 128], f32r, tag="x",
                                            name=f"x_{c}_{mg}_{kt}")
                            nc.sync.dma_start(
                                xs[:],
                                xT[kt * 128:(kt + 1) * 128,
                                   mg * GS * 128:(mg + 1) * GS * 128])
                            if c == 0:
                                nc.tensor.matmul(
                                    ps_t[:], a_sb[:, kt, :], xs[:],
                                    start=(kt == 0), stop=(kt == KT - 1))
                            for mi in range(GS):
                                xsl = xs[:, mi * 128:(mi + 1) * 128]
                                for h in range(nh):
                                    nc.tensor.matmul(
                                        pss[mi][h][:], xsl, wtiles[kt][h][:],
                                        start=(kt == 0), stop=False)
                        if c == 0:
                            tT = tpool.tile([ER, 512], f32r, tag="tT",
                                            name=f"tT_{mg}")
                            nc.vector.tensor_copy(tT[:], ps_t[:])
                            tT_tiles[mg] = tT
                        for mi in range(GS):
                            m = mg * GS + mi
                            if c != 0:
                                tsl = tT_tiles[m // 4][:, (m % 4) * 128:
                                                       (m % 4) * 128 + 128]
                                for h in range(nh):
                                    nc.tensor.matmul(
                                        pss[mi][h][:], tsl, bh[h][:],
                                        start=False, stop=True)
                            for h in range(nh):
                                ev = epool.tile([128, 512], f32, tag="ev",
                                                name=f"ev_{c}_{m}_{h}")
                                if has_bias:
                                    nc.vector.tensor_add(
                                        ev[:], pss[mi][h][:],
                                        bias_all[(col0 // 512) + h][:])
                                elif h == 1:
                                    # spread psum eviction across ACT too:
                                    # frees bank slots ~2x faster at group
                                    # boundaries (all 8 banks per group)
                                    nc.scalar.activation(
                                        ev[:], pss[mi][h][:], act_fn.Copy)
                                else:
                                    nc.vector.tensor_copy(ev[:],
                                                          pss[mi][h][:])
                                nc.sync.dma_start(
                                    out[m * 128:(m + 1) * 128,
                                        col0 + h * 512:col0 + (h + 1) * 512],
                                    ev[:])
                    if c == 0:
                        # gate computation (needs the AllReduce result,
                        # which has landed by now on every core)
                        emit_part_b(mps)
                        b_scl0 = bpool.tile([ER, 512], f32r, tag="bscl",
                                            name="bscl_0_0")
                        nc.vector.tensor_scalar_mul(b_scl0[:], bstg0[0][:],
                                                    gate32[:, 0:1])
                        bh[0] = b_scl0
                        # deferred chunk-0 lora, accumulated via SWDGE
                        for m in range(MT):
                            tsl = tT_tiles[m // 4][:, (m % 4) * 128:
                                                   (m % 4) * 128 + 128]
                            lp = mps.tile([128, 512], f32, tag="ps",
                                          name=f"lp_{m}")
                            nc.tensor.matmul(lp[:], tsl, bh[0][:],
                                             start=True, stop=True)
                            lev = epool.tile([128, 512], f32, tag="ev",
                                             name=f"lev_{m}")
                            nc.vector.tensor_copy(lev[:], lp[:])
                            nc.gpsimd.dma_start(
                                out[m * 128:(m + 1) * 128, 0:512], lev[:],
                                accum_op=alu.add)

    nc.compile()
    return nc


def kernel(**inputs) -> np.ndarray:
    _install_profile_hook()

    x = np.asarray(inputs["x"], dtype=np.float32)
    expert_scores = np.asarray(inputs["expert_scores"], dtype=np.float32)
    W_base = np.asarray(inputs["W_base"], dtype=np.float32)
    b_base = np.asarray(inputs["b_base"], dtype=np.float32)
    gating_W = np.asarray(inputs["gating_W"], dtype=np.float32)
    W_r = np.asarray(inputs["W_r"], dtype=np.float32)
    lora_A = np.asarray(inputs["lora_A"], dtype=np.float32)
    lora_B = np.asarray(inputs["lora_B"], dtype=np.float32)
    module_idx = int(np.asarray(inputs["module_idx"]))
    k = int(np.asarray(inputs["k"]))

    has_bias = bool(np.any(b_base != 0.0))
    key = (k, module_idx, has_bias)
    if key not in _PROGRAM_CACHE:
        _PROGRAM_CACHE[key] = _build_program(k, module_idx, has_bias)
    nc = _PROGRAM_CACHE[key]

    # --- host-side layout prep (transposes/slices/rounding only) --------
    WbT_np = _round_tf32(W_base.T)                       # [D, D]
    WrT_np = _round_tf32(W_r.T)                          # [D, NMOD]
    A_np = _round_tf32(lora_A.reshape(ER, D).T)          # [D, ER]
    B_np = np.ascontiguousarray(
        lora_B.transpose(0, 2, 1).reshape(ER, D))        # [ER, D] fp32
    scores_f_np = np.ascontiguousarray(
        expert_scores.T.reshape(1, E * B))               # [1, E*B]
    b_row_np = b_base.reshape(1, D)
    pooled = x[:, -1, :]                                 # [B, D]

    in_maps = []
    for c in range(N_CORES):
        msel_np = np.zeros((ER, E, B), dtype=np.float32)
        for p in range(ER):
            msel_np[p, p // R, c] = 1.0
        msel_np = msel_np.reshape(ER, E * B)
        in_maps.append({
            "xT": _round_tf32(x[c].T),
            "WbT": WbT_np,
            "gw": _round_tf32(gating_W[:, c * DSH:(c + 1) * DSH]),
            "WrT": WrT_np,
            "pooledT": np.ascontiguousarray(
                pooled[:, c * DSH:(c + 1) * DSH].T),
            "scores_f": scores_f_np,
            "A_rhs": A_np,
            "B_cat": B_np,
            "b_row": b_row_np,
            "msel": msel_np,
        })

    from concourse.bass_utils import run_bass_kernel_spmd
    res = run_bass_kernel_spmd(nc, in_maps, core_ids=list(range(N_CORES)))
    return np.stack([res.results[c]["out"] for c in range(N_CORES)], axis=0)


if __name__ == "__main__":
    rng = np.random.default_rng(0)
    demo = {
        "x": (rng.standard_normal((B, L, D)) * 0.02).astype(np.float32),
        "expert_scores": rng.random((B, E), dtype=np.float32),
        "W_base": (rng.standard_normal((D, D)) * 0.02).astype(np.float32),
        "b_base": np.zeros(D, np.float32),
        "gating_W": (rng.standard_normal((D, D)) * 0.02).astype(np.float32),
        "W_r": (rng.standard_normal((NMOD, D)) * 0.02).astype(np.float32),
        "lora_A": (rng.standard_normal((E, R, D)) * 0.02).astype(np.float32),
        "lora_B": (rng.standard_normal((E, D, R)) * 0.02).astype(np.float32),
        "module_idx": 0,
        "k": 2,
    }
    y = kernel(**demo)
    print("out", y.shape, y.dtype, float(np.abs(y).max()))



# revision 3
# speedup vs baseline: 1.2919x; 1.2919x over previous
"""DynaLoRALinear Trainium2 kernel.

Data-parallel over batch B across 8 NeuronCores (one sample per core).
Per core:
  - router:  logits = pooled @ (W_r @ gating_W).T with the weight product
    folded on host into WeT [D, NMOD]; every core holds the full pooled
    [D, B] (128 KB) and computes all B logits locally -> no collective.
  - gate weights from expert_scores ranks + module_prob>0.5 branch select,
    computed up front so chunk-0 LoRA applies inline (no deferred pass).
  - base:    out = x_b @ W_base.T + b_base   (bf16 matmuls, fp32 PSUM)
  - lora:    t = x_b @ A_cat.T (fused into chunk-0 k-loop), then
             out += t @ (B_cat * gate).T
All matmul operands are bf16 (1 cyc/row on PE, half the DMA bytes of
f32r) at ~2e-3 scale-relative absmax error vs the 2e-2 gate. W_base
streams in 5 column chunks; each chunk's tiles are prefetched during the
previous chunk's compute so the PE never idles at chunk boundaries
(idle >~5us trips the HAM clock gate down to 4/8).
"""

import sys
import types

import numpy as np

B, L, D, E, R, NMOD = 8, 2048, 4096, 4, 8, 7
N_CORES = 8
ER = E * R          # 32
KT = D // 128       # 32 k-tiles
MT = L // 128       # 16 m-tiles
GS = 4              # m-tiles per group
NG = MT // GS       # 4 groups
CHUNKS = [(0, 512), (512, 1024), (1536, 1024), (2560, 1024), (3584, 512)]


def _round_bf16(a) -> np.ndarray:
    import ml_dtypes
    return np.ascontiguousarray(a, dtype=np.float32).astype(ml_dtypes.bfloat16)


def _install_profile_hook():
    """Make bass_utils' trace path importable (no-op if already present)."""
    try:
        import antenv.axon_hooks  # noqa: F401
        return
    except ImportError:
        pass
    try:
        import antenv
    except ImportError:
        return
    mod = types.ModuleType("antenv.axon_hooks")
    mod._hook = None
    mod.set_axon_ntff_profile_hook = lambda h: setattr(mod, "_hook", h)
    mod.get_axon_ntff_profile_hook = lambda: mod._hook
    sys.modules["antenv.axon_hooks"] = mod
    antenv.axon_hooks = mod
    try:
        from trn_agent_boot.trn_boot import _ntff_profile_via_ctypes
        hook = _ntff_profile_via_ctypes("/opt/axon/libaxon_pjrt.so")
        if hook is not None:
            mod.set_axon_ntff_profile_hook(hook)
    except Exception:
        pass


_PROGRAM_CACHE = {}


def _build_program(k: int, module_idx: int, has_bias: bool):
    import concourse.mybir as mybir
    import concourse.tile as tile
    from concourse import bacc
    from concourse.masks import make_identity

    f32 = mybir.dt.float32
    bf16 = mybir.dt.bfloat16
    alu = mybir.AluOpType
    act_fn = mybir.ActivationFunctionType

    k_lo = max(1, k // 2)

    nc = bacc.Bacc("TRN2", target_bir_lowering=False, debug=False,
                   num_devices=N_CORES)

    # --- DRAM I/O -------------------------------------------------------
    xT = nc.dram_tensor("xT", [D, L], bf16, kind="ExternalInput")
    WbT = nc.dram_tensor("WbT", [D, D], bf16, kind="ExternalInput")
    WeT = nc.dram_tensor("WeT", [D, NMOD], bf16, kind="ExternalInput")
    pooledT = nc.dram_tensor("pooledT", [D, B], bf16, kind="ExternalInput")
    scores_f = nc.dram_tensor("scores_f", [1, E * B], f32,
                              kind="ExternalInput")
    A_rhs = nc.dram_tensor("A_rhs", [D, ER], bf16, kind="ExternalInput")
    B_cat = nc.dram_tensor("B_cat", [ER, D], f32, kind="ExternalInput")
    b_row = nc.dram_tensor("b_row", [1, D], f32, kind="ExternalInput")
    msel = nc.dram_tensor("msel", [ER, E * B], f32, kind="ExternalInput")
    out = nc.dram_tensor("out", [L, D], f32, kind="ExternalOutput")

    with tile.TileContext(nc) as tc:
        with (
            tc.tile_pool(name="const", bufs=1) as const_pool,
            tc.tile_pool(name="gatep", bufs=1) as gate_pool,
            tc.tile_pool(name="rsb", bufs=1) as rsb,
            tc.tile_pool(name="wpool", bufs=130) as wpool,
            tc.tile_pool(name="xpool", bufs=10) as xpool,
            tc.tile_pool(name="apool", bufs=1) as apool,
            tc.tile_pool(name="tpool", bufs=4) as tpool,
            tc.tile_pool(name="bpool", bufs=2) as bpool,
            tc.tile_pool(name="biasp", bufs=(D // 512 if has_bias else 1))
                as biasp,
            tc.tile_pool(name="epool", bufs=4) as epool,
        ):
            ident = const_pool.tile([128, 128], f32)
            make_identity(nc, ident)
            gate32 = gate_pool.tile([ER, 1], f32)

            # chunk-0 W: first 8 k-tiles go out before anything else so the
            # PE can start as early as possible.
            wtiles_cur = []
            for kt in range(8):
                wt = wpool.tile([128, 512], bf16, tag="w", name=f"w_0_{kt}")
                nc.sync.dma_start(wt[:], WbT[kt * 128:(kt + 1) * 128, 0:512])
                wtiles_cur.append([wt])

            # small inputs (router, gate, lora operands)
            we_sb = rsb.tile([128, KT, NMOD], bf16)
            nc.sync.dma_start(
                we_sb[:], WeT[:].rearrange("(a p) m -> p a m", p=128))
            pt_sb = rsb.tile([128, KT, B], bf16)
            nc.sync.dma_start(
                pt_sb[:], pooledT[:].rearrange("(a p) m -> p a m", p=128))
            sc = rsb.tile([1, E * B], f32)
            nc.sync.dma_start(sc[:], scores_f[:])
            msel_sb = rsb.tile([ER, E * B], f32)
            nc.sync.dma_start(msel_sb[:], msel[:])
            a_sb = apool.tile([128, KT, ER], bf16)
            nc.sync.dma_start(
                a_sb[:], A_rhs[:].rearrange("(a p) m -> p a m", p=128))
            ball_stg = bpool.tile([ER, D], f32, tag="bstg")
            nc.sync.dma_start(ball_stg[:], B_cat[:])

            bias_all = []
            if has_bias:
                for hh in range(D // 512):
                    bias_bc = biasp.tile([128, 512], f32, tag="biasbc",
                                         name=f"biasbc_{hh}")
                    nc.sync.dma_start(
                        bias_bc[0:1, :],
                        b_row[:, hh * 512:(hh + 1) * 512])
                    nc.gpsimd.partition_broadcast(bias_bc[:],
                                                  bias_bc[0:1, :])
                    bias_all.append(bias_bc)

            # rest of chunk-0 W
            for kt in range(8, KT):
                wt = wpool.tile([128, 512], bf16, tag="w", name=f"w_0_{kt}")
                nc.sync.dma_start(wt[:], WbT[kt * 128:(kt + 1) * 128, 0:512])
                wtiles_cur.append([wt])

            # ====== router + gate (all local; no collective) ============
            with tc.tile_pool(name="rps", bufs=2, space="PSUM") as rps:
                lg_ps = rps.tile([NMOD, B], f32, tag="ps", name="lgps")
                for kt in range(KT):
                    nc.tensor.matmul(lg_ps[:], we_sb[:, kt, :],
                                     pt_sb[:, kt, :],
                                     start=(kt == 0), stop=(kt == KT - 1))
                lg_sb = rsb.tile([NMOD, B], f32)
                nc.vector.tensor_copy(lg_sb[:], lg_ps[:])

                # expert ranks from scores (independent of router)
                rank = rsb.tile([1, E * B], f32)
                nc.vector.memset(rank[:], 0.0)
                tmp = rsb.tile([1, B], f32)
                for e in range(E):
                    re = rank[:, e * B:(e + 1) * B]
                    se = sc[:, e * B:(e + 1) * B]
                    for e2 in range(E):
                        if e2 == e:
                            continue
                        s2 = sc[:, e2 * B:(e2 + 1) * B]
                        nc.vector.tensor_tensor(tmp[:], s2, se, op=alu.is_gt)
                        nc.vector.tensor_add(re, re, tmp[:])
                        if e2 < e:
                            nc.vector.tensor_tensor(tmp[:], s2, se,
                                                    op=alu.is_equal)
                            nc.vector.tensor_add(re, re, tmp[:])
                w_hi = rsb.tile([1, E * B], f32)
                nc.vector.tensor_scalar(w_hi[:], rank[:], float(k),
                                        1.0 / float(k),
                                        op0=alu.is_lt, op1=alu.mult)
                w_lo = rsb.tile([1, E * B], f32)
                nc.vector.tensor_scalar(w_lo[:], rank[:], float(k_lo),
                                        1.0 / float(k_lo),
                                        op0=alu.is_lt, op1=alu.mult)
                diff = rsb.tile([1, E * B], f32)
                nc.vector.tensor_sub(diff[:], w_hi[:], w_lo[:])

                # softmax -> module_prob>0.5 -> gate -> per-row gate32
                ltp = rps.tile([B, NMOD], f32, tag="ps", name="ltp")
                nc.tensor.transpose(ltp[:], lg_sb[:], ident[0:NMOD, 0:NMOD])
                lt = rsb.tile([B, NMOD], f32)
                nc.vector.tensor_copy(lt[:], ltp[:])
                mx = rsb.tile([B, 1], f32)
                nc.vector.tensor_reduce(out=mx[:], in_=lt[:], op=alu.max,
                                        axis=mybir.AxisListType.X)
                mxn = rsb.tile([B, 1], f32)
                nc.vector.tensor_scalar_mul(mxn[:], mx[:], -1.0)
                ex = rsb.tile([B, NMOD], f32)
                nc.scalar.activation(ex[:], lt[:], act_fn.Exp, bias=mxn[:])
                sm = rsb.tile([B, 1], f32)
                nc.vector.tensor_reduce(out=sm[:], in_=ex[:], op=alu.add,
                                        axis=mybir.AxisListType.X)
                rs = rsb.tile([B, 1], f32)
                nc.vector.reciprocal(rs[:], sm[:])
                p0 = rsb.tile([B, 1], f32)
                nc.vector.tensor_mul(
                    p0[:], ex[:, module_idx:module_idx + 1], rs[:])
                hi = rsb.tile([B, 1], f32)
                nc.vector.tensor_single_scalar(hi[:], p0[:], 0.5, alu.is_gt)
                hp = rps.tile([1, B], f32, tag="ps", name="hp")
                nc.tensor.transpose(hp[:], hi[:], ident[0:B, 0:B])
                hi_row = rsb.tile([1, B], f32)
                nc.vector.tensor_copy(hi_row[:], hp[:])
                gate = rsb.tile([1, E * B], f32)
                for e in range(E):
                    nc.vector.tensor_mul(gate[:, e * B:(e + 1) * B],
                                         diff[:, e * B:(e + 1) * B],
                                         hi_row[:])
                nc.vector.tensor_add(gate[:], gate[:], w_lo[:])
                gateb = rsb.tile([ER, E * B], f32)
                nc.gpsimd.partition_broadcast(gateb[:], gate[:])
                g32m = rsb.tile([ER, E * B], f32)
                nc.vector.tensor_tensor(g32m[:], gateb[:], msel_sb[:],
                                        op=alu.mult)
                nc.vector.tensor_reduce(out=gate32[:], in_=g32m[:],
                                        op=alu.add,
                                        axis=mybir.AxisListType.X)

            # scale the whole B_cat by this core's gate once, in bf16
            b_all = bpool.tile([ER, D], bf16, tag="ball")
            nc.vector.tensor_scalar_mul(b_all[:], ball_stg[:],
                                        gate32[:, 0:1])

            # ============== main: base + lora ==========================
            with tc.tile_pool(name="mps", bufs=8, space="PSUM") as mps:
                tT_tiles = [None] * NG
                for c, (col0, width) in enumerate(CHUNKS):
                    nh = width // 512
                    bh = [b_all[:, col0 + h * 512:col0 + (h + 1) * 512]
                          for h in range(nh)]
                    if c + 1 < len(CHUNKS):
                        ncol0, nwidth = CHUNKS[c + 1]
                        nnh = nwidth // 512
                        wtiles_next = [[None] * nnh for _ in range(KT)]
                    for mg in range(NG):
                        pss = []
                        for mi in range(GS):
                            row = []
                            for h in range(nh):
                                ps = mps.tile([128, 512], f32, tag="ps",
                                              name=f"ps_{c}_{mg}_{mi}_{h}")
                                row.append(ps)
                            pss.append(row)
                        if c == 0:
                            ps_t = mps.tile([ER, 512], f32, tag="ps",
                                            name=f"pst_{mg}")
                        for kt in range(KT):
                            xs = xpool.tile([128, GS * 128], bf16, tag="x",
                                            name=f"x_{c}_{mg}_{kt}")
                            nc.sync.dma_start(
                                xs[:],
                                xT[kt * 128:(kt + 1) * 128,
                                   mg * 512:(mg + 1) * 512])
                            if c == 0:
                                nc.tensor.matmul(
                                    ps_t[:], a_sb[:, kt, :], xs[:],
                                    start=(kt == 0), stop=(kt == KT - 1))
                            for mi in range(GS):
                                xsl = xs[:, mi * 128:(mi + 1) * 128]
                                for h in range(nh):
                                    nc.tensor.matmul(
                                        pss[mi][h][:], xsl,
                                        wtiles_cur[kt][h][:],
                                        start=(kt == 0), stop=False)
                        if c == 0:
                            tT = tpool.tile([ER, 512], bf16, tag="tT",
                                            name=f"tT_{mg}")
                            nc.vector.tensor_copy(tT[:], ps_t[:])
                            tT_tiles[mg] = tT
                        # prefetch 1/NG of the next chunk's W tiles while
                        # this group's lora/evictions run
                        if c + 1 < len(CHUNKS):
                            for kt in range(mg * (KT // NG),
                                            (mg + 1) * (KT // NG)):
                                for h in range(nnh):
                                    wt = wpool.tile(
                                        [128, 512], bf16, tag="w",
                                        name=f"w_{c + 1}_{kt}_{h}")
                                    nc.sync.dma_start(
                                        wt[:],
                                        WbT[kt * 128:(kt + 1) * 128,
                                            ncol0 + h * 512:
                                            ncol0 + (h + 1) * 512])
                                    wtiles_next[kt][h] = wt
                        for mi in range(GS):
                            m = mg * GS + mi
                            tsl = tT_tiles[m // 4][:, (m % 4) * 128:
                                                   (m % 4) * 128 + 128]
                            for h in range(nh):
                                nc.tensor.matmul(pss[mi][h][:], tsl, bh[h],
                                                 start=False, stop=True)
                            for h in range(nh):
                                ev = epool.tile([128, 512], f32, tag="ev",
                                                name=f"ev_{c}_{m}_{h}")
                                if has_bias:
                                    nc.vector.tensor_add(
                                        ev[:], pss[mi][h][:],
                                        bias_all[(col0 // 512) + h][:])
                                elif h == 1:
                                    # spread psum eviction across ACT too
                                    nc.scalar.activation(
                                        ev[:], pss[mi][h][:], act_fn.Copy)
                                else:
                                    nc.vector.tensor_copy(ev[:],
                                                          pss[mi][h][:])
                                nc.sync.dma_start(
                                    out[m * 128:(m + 1) * 128,
                                        col0 + h * 512:col0 + (h + 1) * 512],
                                    ev[:])
                    if c + 1 < len(CHUNKS):
                        wtiles_cur = wtiles_next

    nc.compile()
    return nc


def kernel(**inputs) -> np.ndarray:
    _install_profile_hook()

    x = np.asarray(inputs["x"], dtype=np.float32)
    expert_scores = np.asarray(inputs["expert_scores"], dtype=np.float32)
    W_base = np.asarray(inputs["W_base"], dtype=np.float32)
    b_base = np.asarray(inputs["b_base"], dtype=np.float32)
    gating_W = np.asarray(inputs["gating_W"], dtype=np.float32)
    W_r = np.asarray(inputs["W_r"], dtype=np.float32)
    lora_A = np.asarray(inputs["lora_A"], dtype=np.float32)
    lora_B = np.asarray(inputs["lora_B"], dtype=np.float32)
    module_idx = int(np.asarray(inputs["module_idx"]))
    k = int(np.asarray(inputs["k"]))

    has_bias = bool(np.any(b_base != 0.0))
    key = (k, module_idx, has_bias)
    if key not in _PROGRAM_CACHE:
        _PROGRAM_CACHE[key] = _build_program(k, module_idx, has_bias)
    nc = _PROGRAM_CACHE[key]

    # --- host-side layout prep (transposes/slices/rounding only; the
    # router weight product W_r @ gating_W is a fixed function of weights)
    WbT_np = _round_bf16(W_base.T)                       # [D, D]
    WeT_np = _round_bf16((W_r @ gating_W).T)             # [D, NMOD]
    A_np = _round_bf16(lora_A.reshape(ER, D).T)          # [D, ER]
    B_np = np.ascontiguousarray(
        lora_B.transpose(0, 2, 1).reshape(ER, D))        # [ER, D] fp32
    scores_f_np = np.ascontiguousarray(
        expert_scores.T.reshape(1, E * B))               # [1, E*B]
    b_row_np = b_base.reshape(1, D)
    pooledT_np = _round_bf16(x[:, -1, :].T)              # [D, B]

    in_maps = []
    for c in range(N_CORES):
        msel_np = np.zeros((ER, E, B), dtype=np.float32)
        for p in range(ER):
            msel_np[p, p // R, c] = 1.0
        msel_np = msel_np.reshape(ER, E * B)
        in_maps.append({
            "xT": _round_bf16(x[c].T),
            "WbT": WbT_np,
            "WeT": WeT_np,
            "pooledT": pooledT_np,
            "scores_f": scores_f_np,
            "A_rhs": A_np,
            "B_cat": B_np,
            "b_row": b_row_np,
            "msel": msel_np,
        })

    from concourse.bass_utils import run_bass_kernel_spmd
    res = run_bass_kernel_spmd(nc, in_maps, core_ids=list(range(N_CORES)))
    return np.stack([res.results[c]["out"] for c in range(N_CORES)], axis=0)


if __name__ == "__main__":
    rng = np.random.default_rng(0)
    demo = {
        "x": (rng.standard_normal((B, L, D)) * 0.02).astype(np.float32),
        "expert_scores": rng.random((B, E), dtype=np.float32),
        "W_base": (rng.standard_normal((D, D)) * 0.02).astype(np.float32),
        "b_base": np.zeros(D, np.float32),
        "gating_W": (rng.standard_normal((D, D)) * 0.02).astype(np.float32),
        "W_r": (rng.standard_normal((NMOD, D)) * 0.02).astype(np.float32),
        "lora_A": (rng.standard_normal((E, R, D)) * 0.02).astype(np.float32),
        "lora_B": (rng.standard_normal((E, D, R)) * 0.02).astype(np.float32),
        "module_idx": 0,
        "k": 2,
    }
    y = kernel(**demo)
    print("out", y.shape, y.dtype, float(np.abs(y).max()))


# revision 8
# speedup vs baseline: 1.3132x; 1.0165x over previous
"""DynaLoRALinear Trainium2 kernel.

Data-parallel over batch B across 8 NeuronCores (one sample per core).
Per core:
  - router:  logits = pooled @ (W_r @ gating_W).T with the weight product
    folded on host into WeT [D, NMOD]; every core holds the full pooled
    [D, B] (128 KB) and computes all B logits locally -> no collective.
  - gate weights from expert_scores ranks + module_prob>0.5 branch select,
    computed up front so chunk-0 LoRA applies inline (no deferred pass).
  - base:    out = x_b @ W_base.T + b_base   (bf16 matmuls, fp32 PSUM)
  - lora:    t = x_b @ A_cat.T (fused into chunk-0 k-loop), then
             out += t @ (B_cat * gate).T
All matmul operands are bf16 (1 cyc/row on PE, half the DMA bytes of
f32r) at ~2e-3 scale-relative absmax error vs the 2e-2 gate. W_base
streams in 5 column chunks; each chunk's tiles are prefetched during the
previous chunk's compute so the PE never idles at chunk boundaries
(idle >~5us trips the HAM clock gate down to 4/8).
"""

import sys
import types

import numpy as np

B, L, D, E, R, NMOD = 8, 2048, 4096, 4, 8, 7
N_CORES = 8
ER = E * R          # 32
KT = D // 128       # 32 k-tiles
MT = L // 128       # 16 m-tiles
GS = 4              # m-tiles per group
NG = MT // GS       # 4 groups
CHUNKS = [(0, 512), (512, 1024), (1536, 1024), (2560, 1024), (3584, 512)]


def _round_bf16(a) -> np.ndarray:
    import ml_dtypes
    return np.ascontiguousarray(a, dtype=np.float32).astype(ml_dtypes.bfloat16)


def _install_profile_hook():
    """Make bass_utils' trace path importable (no-op if already present)."""
    try:
        import antenv.axon_hooks  # noqa: F401
        return
    except ImportError:
        pass
    try:
        import antenv
    except ImportError:
        return
    mod = types.ModuleType("antenv.axon_hooks")
    mod._hook = None
    mod.set_axon_ntff_profile_hook = lambda h: setattr(mod, "_hook", h)
    mod.get_axon_ntff_profile_hook = lambda: mod._hook
    sys.modules["antenv.axon_hooks"] = mod
    antenv.axon_hooks = mod
    try:
        from trn_agent_boot.trn_boot import _ntff_profile_via_ctypes
        hook = _ntff_profile_via_ctypes("/opt/axon/libaxon_pjrt.so")
        if hook is not None:
            mod.set_axon_ntff_profile_hook(hook)
    except Exception:
        pass


_PROGRAM_CACHE = {}


def _build_program(k: int, module_idx: int, has_bias: bool):
    import concourse.mybir as mybir
    import concourse.tile as tile
    from concourse import bacc
    from concourse.masks import make_identity

    f32 = mybir.dt.float32
    bf16 = mybir.dt.bfloat16
    alu = mybir.AluOpType
    act_fn = mybir.ActivationFunctionType

    k_lo = max(1, k // 2)

    nc = bacc.Bacc("TRN2", target_bir_lowering=False, debug=False,
                   num_devices=N_CORES)

    # --- DRAM I/O -------------------------------------------------------
    xT = nc.dram_tensor("xT", [D, L], bf16, kind="ExternalInput")
    WbT = nc.dram_tensor("WbT", [D, D], bf16, kind="ExternalInput")
    WeT = nc.dram_tensor("WeT", [D, NMOD], bf16, kind="ExternalInput")
    pooledT = nc.dram_tensor("pooledT", [D, B], bf16, kind="ExternalInput")
    scores_f = nc.dram_tensor("scores_f", [1, E * B], f32,
                              kind="ExternalInput")
    A_rhs = nc.dram_tensor("A_rhs", [D, ER], bf16, kind="ExternalInput")
    B_cat = nc.dram_tensor("B_cat", [ER, D], f32, kind="ExternalInput")
    b_row = nc.dram_tensor("b_row", [1, D], f32, kind="ExternalInput")
    msel = nc.dram_tensor("msel", [ER, E * B], f32, kind="ExternalInput")
    out = nc.dram_tensor("out", [L, D], f32, kind="ExternalOutput")

    with tile.TileContext(nc) as tc:
        with (
            tc.tile_pool(name="const", bufs=1) as const_pool,
            tc.tile_pool(name="gatep", bufs=1) as gate_pool,
            tc.tile_pool(name="rsb", bufs=1) as rsb,
            tc.tile_pool(name="wpool", bufs=130) as wpool,
            tc.tile_pool(name="xpool", bufs=10) as xpool,
            tc.tile_pool(name="apool", bufs=1) as apool,
            tc.tile_pool(name="tpool", bufs=4) as tpool,
            tc.tile_pool(name="bpool", bufs=2) as bpool,
            tc.tile_pool(name="biasp", bufs=(D // 512 if has_bias else 1))
                as biasp,
            tc.tile_pool(name="epool", bufs=4) as epool,
        ):
            ident = const_pool.tile([128, 128], f32)
            make_identity(nc, ident)
            gate32 = gate_pool.tile([ER, 1], f32)

            # small inputs (router, gate, lora operands); chunk-0 W tiles
            # are interleaved with group-0's x stream in the main loop so
            # the PE starts on the first k-tile as soon as it lands.
            wtiles_cur = [None] * KT
            we_sb = rsb.tile([128, KT, NMOD], bf16)
            nc.sync.dma_start(
                we_sb[:], WeT[:].rearrange("(a p) m -> p a m", p=128))
            pt_sb = rsb.tile([128, KT, B], bf16)
            nc.sync.dma_start(
                pt_sb[:], pooledT[:].rearrange("(a p) m -> p a m", p=128))
            sc = rsb.tile([1, E * B], f32)
            nc.sync.dma_start(sc[:], scores_f[:])
            msel_sb = rsb.tile([ER, E * B], f32)
            nc.sync.dma_start(msel_sb[:], msel[:])
            a_sb = apool.tile([128, KT, ER], bf16)
            nc.sync.dma_start(
                a_sb[:], A_rhs[:].rearrange("(a p) m -> p a m", p=128))
            ball_stg = bpool.tile([ER, D], f32, tag="bstg")
            nc.sync.dma_start(ball_stg[:], B_cat[:])

            bias_all = []
            if has_bias:
                for hh in range(D // 512):
                    bias_bc = biasp.tile([128, 512], f32, tag="biasbc",
                                         name=f"biasbc_{hh}")
                    nc.sync.dma_start(
                        bias_bc[0:1, :],
                        b_row[:, hh * 512:(hh + 1) * 512])
                    nc.gpsimd.partition_broadcast(bias_bc[:],
                                                  bias_bc[0:1, :])
                    bias_all.append(bias_bc)

            # ====== router + gate (all local; no collective) ============
            with tc.tile_pool(name="rps", bufs=2, space="PSUM") as rps:
                lg_ps = rps.tile([NMOD, B], f32, tag="ps", name="lgps")
                for kt in range(KT):
                    nc.tensor.matmul(lg_ps[:], we_sb[:, kt, :],
                                     pt_sb[:, kt, :],
                                     start=(kt == 0), stop=(kt == KT - 1))
                lg_sb = rsb.tile([NMOD, B], f32)
                nc.vector.tensor_copy(lg_sb[:], lg_ps[:])

                # expert ranks from scores (independent of router)
                rank = rsb.tile([1, E * B], f32)
                nc.vector.memset(rank[:], 0.0)
                tmp = rsb.tile([1, B], f32)
                for e in range(E):
                    re = rank[:, e * B:(e + 1) * B]
                    se = sc[:, e * B:(e + 1) * B]
                    for e2 in range(E):
                        if e2 == e:
                            continue
                        s2 = sc[:, e2 * B:(e2 + 1) * B]
                        nc.vector.tensor_tensor(tmp[:], s2, se, op=alu.is_gt)
                        nc.vector.tensor_add(re, re, tmp[:])
                        if e2 < e:
                            nc.vector.tensor_tensor(tmp[:], s2, se,
                                                    op=alu.is_equal)
                            nc.vector.tensor_add(re, re, tmp[:])
                w_hi = rsb.tile([1, E * B], f32)
                nc.vector.tensor_scalar(w_hi[:], rank[:], float(k),
                                        1.0 / float(k),
                                        op0=alu.is_lt, op1=alu.mult)
                w_lo = rsb.tile([1, E * B], f32)
                nc.vector.tensor_scalar(w_lo[:], rank[:], float(k_lo),
                                        1.0 / float(k_lo),
                                        op0=alu.is_lt, op1=alu.mult)
                diff = rsb.tile([1, E * B], f32)
                nc.vector.tensor_sub(diff[:], w_hi[:], w_lo[:])

                # softmax -> module_prob>0.5 -> gate -> per-row gate32
                ltp = rps.tile([B, NMOD], f32, tag="ps", name="ltp")
                nc.tensor.transpose(ltp[:], lg_sb[:], ident[0:NMOD, 0:NMOD])
                lt = rsb.tile([B, NMOD], f32)
                nc.vector.tensor_copy(lt[:], ltp[:])
                mx = rsb.tile([B, 1], f32)
                nc.vector.tensor_reduce(out=mx[:], in_=lt[:], op=alu.max,
                                        axis=mybir.AxisListType.X)
                mxn = rsb.tile([B, 1], f32)
                nc.vector.tensor_scalar_mul(mxn[:], mx[:], -1.0)
                ex = rsb.tile([B, NMOD], f32)
                nc.scalar.activation(ex[:], lt[:], act_fn.Exp, bias=mxn[:])
                sm = rsb.tile([B, 1], f32)
                nc.vector.tensor_reduce(out=sm[:], in_=ex[:], op=alu.add,
                                        axis=mybir.AxisListType.X)
                rs = rsb.tile([B, 1], f32)
                nc.vector.reciprocal(rs[:], sm[:])
                p0 = rsb.tile([B, 1], f32)
                nc.vector.tensor_mul(
                    p0[:], ex[:, module_idx:module_idx + 1], rs[:])
                hi = rsb.tile([B, 1], f32)
                nc.vector.tensor_single_scalar(hi[:], p0[:], 0.5, alu.is_gt)
                hp = rps.tile([1, B], f32, tag="ps", name="hp")
                nc.tensor.transpose(hp[:], hi[:], ident[0:B, 0:B])
                hi_row = rsb.tile([1, B], f32)
                nc.vector.tensor_copy(hi_row[:], hp[:])
                gate = rsb.tile([1, E * B], f32)
                for e in range(E):
                    nc.vector.tensor_mul(gate[:, e * B:(e + 1) * B],
                                         diff[:, e * B:(e + 1) * B],
                                         hi_row[:])
                nc.vector.tensor_add(gate[:], gate[:], w_lo[:])
                gateb = rsb.tile([ER, E * B], f32)
                nc.gpsimd.partition_broadcast(gateb[:], gate[:])
                g32m = rsb.tile([ER, E * B], f32)
                nc.vector.tensor_tensor(g32m[:], gateb[:], msel_sb[:],
                                        op=alu.mult)
                nc.vector.tensor_reduce(out=gate32[:], in_=g32m[:],
                                        op=alu.add,
                                        axis=mybir.AxisListType.X)

            # scale the whole B_cat by this core's gate once, in bf16
            b_all = bpool.tile([ER, D], bf16, tag="ball")
            nc.vector.tensor_scalar_mul(b_all[:], ball_stg[:],
                                        gate32[:, 0:1])

            # ============== main: base + lora ==========================
            with tc.tile_pool(name="mps", bufs=8, space="PSUM") as mps:
                tT_tiles = [None] * NG
                for c, (col0, width) in enumerate(CHUNKS):
                    nh = width // 512
                    bh = [b_all[:, col0 + h * 512:col0 + (h + 1) * 512]
                          for h in range(nh)]
                    if c + 1 < len(CHUNKS):
                        ncol0, nwidth = CHUNKS[c + 1]
                        nnh = nwidth // 512
                        wtiles_next = [[None] * nnh for _ in range(KT)]
                    for mg in range(NG):
                        pss = []
                        for mi in range(GS):
                            row = []
                            for h in range(nh):
                                ps = mps.tile([128, 512], f32, tag="ps",
                                              name=f"ps_{c}_{mg}_{mi}_{h}")
                                row.append(ps)
                            pss.append(row)
                        if c == 0:
                            ps_t = mps.tile([ER, 512], f32, tag="ps",
                                            name=f"pst_{mg}")
                        for kt in range(KT):
                            if c == 0 and mg == 0:
                                wt = wpool.tile([128, 512], bf16, tag="w",
                                                name=f"w_0_{kt}")
                                nc.sync.dma_start(
                                    wt[:],
                                    WbT[kt * 128:(kt + 1) * 128, 0:512])
                                wtiles_cur[kt] = [wt]
                            xs = xpool.tile([128, GS * 128], bf16, tag="x",
                                            name=f"x_{c}_{mg}_{kt}")
                            nc.sync.dma_start(
                                xs[:],
                                xT[kt * 128:(kt + 1) * 128,
                                   mg * 512:(mg + 1) * 512])
                            if c == 0:
                                nc.tensor.matmul(
                                    ps_t[:], a_sb[:, kt, :], xs[:],
                                    start=(kt == 0), stop=(kt == KT - 1))
                            for mi in range(GS):
                                xsl = xs[:, mi * 128:(mi + 1) * 128]
                                for h in range(nh):
                                    nc.tensor.matmul(
                                        pss[mi][h][:], xsl,
                                        wtiles_cur[kt][h][:],
                                        start=(kt == 0), stop=False)
                        if c == 0:
                            tT = tpool.tile([ER, 512], bf16, tag="tT",
                                            name=f"tT_{mg}")
                            nc.vector.tensor_copy(tT[:], ps_t[:])
                            tT_tiles[mg] = tT
                        # prefetch 1/NG of the next chunk's W tiles while
                        # this group's lora/evictions run
                        if c + 1 < len(CHUNKS):
                            for kt in range(mg * (KT // NG),
                                            (mg + 1) * (KT // NG)):
                                for h in range(nnh):
                                    wt = wpool.tile(
                                        [128, 512], bf16, tag="w",
                                        name=f"w_{c + 1}_{kt}_{h}")
                                    nc.sync.dma_start(
                                        wt[:],
                                        WbT[kt * 128:(kt + 1) * 128,
                                            ncol0 + h * 512:
                                            ncol0 + (h + 1) * 512])
                                    wtiles_next[kt][h] = wt
                        for mi in range(GS):
                            m = mg * GS + mi
                            tsl = tT_tiles[m // 4][:, (m % 4) * 128:
                                                   (m % 4) * 128 + 128]
                            for h in range(nh):
                                nc.tensor.matmul(pss[mi][h][:], tsl, bh[h],
                                                 start=False, stop=True)
                            for h in range(nh):
                                ev = epool.tile([128, 512], f32, tag="ev",
                                                name=f"ev_{c}_{m}_{h}")
                                # evict + store from the producing engine's
                                # queue so the sync queue carries only loads
                                # and never blocks on compute progress
                                if has_bias:
                                    nc.vector.tensor_add(
                                        ev[:], pss[mi][h][:],
                                        bias_all[(col0 // 512) + h][:])
                                elif h == 1:
                                    # spread psum eviction across ACT too
                                    nc.scalar.activation(
                                        ev[:], pss[mi][h][:], act_fn.Copy)
                                else:
                                    nc.vector.tensor_copy(ev[:],
                                                          pss[mi][h][:])
                                nc.scalar.dma_start(
                                    out[m * 128:(m + 1) * 128,
                                        col0 + h * 512:col0 + (h + 1) * 512],
                                    ev[:])
                    if c + 1 < len(CHUNKS):
                        wtiles_cur = wtiles_next

    nc.compile()
    return nc


def kernel(**inputs) -> np.ndarray:
    _install_profile_hook()

    x = np.asarray(inputs["x"], dtype=np.float32)
    expert_scores = np.asarray(inputs["expert_scores"], dtype=np.float32)
    W_base = np.asarray(inputs["W_base"], dtype=np.float32)
    b_base = np.asarray(inputs["b_base"], dtype=np.float32)
    gating_W = np.asarray(inputs["gating_W"], dtype=np.float32)
    W_r = np.asarray(inputs["W_r"], dtype=np.float32)
    lora_A = np.asarray(inputs["lora_A"], dtype=np.float32)
    lora_B = np.asarray(inputs["lora_B"], dtype=np.float32)
    module_idx = int(np.asarray(inputs["module_idx"]))
    k = int(np.asarray(inputs["k"]))

    has_bias = bool(np.any(b_base != 0.0))
    key = (k, module_idx, has_bias)
    if key not in _PROGRAM_CACHE:
        _PROGRAM_CACHE[key] = _build_program(k, module_idx, has_bias)
    nc = _PROGRAM_CACHE[key]

    # --- host-side layout prep (transposes/slices/rounding only; the
    # router weight product W_r @ gating_W is a fixed function of weights)
    WbT_np = _round_bf16(W_base.T)                       # [D, D]
    WeT_np = _round_bf16((W_r @ gating_W).T)             # [D, NMOD]
    A_np = _round_bf16(lora_A.reshape(ER, D).T)          # [D, ER]
    B_np = np.ascontiguousarray(
        lora_B.transpose(0, 2, 1).reshape(ER, D))        # [ER, D] fp32
    scores_f_np = np.ascontiguousarray(
        expert_scores.T.reshape(1, E * B))               # [1, E*B]
    b_row_np = b_base.reshape(1, D)
    pooledT_np = _round_bf16(x[:, -1, :].T)              # [D, B]

    in_maps = []
    for c in range(N_CORES):
        msel_np = np.zeros((ER, E, B), dtype=np.float32)
        for p in range(ER):
            msel_np[p, p // R, c] = 1.0
        msel_np = msel_np.reshape(ER, E * B)
        in_maps.append({
            "xT": _round_bf16(x[c].T),
            "WbT": WbT_np,
            "WeT": WeT_np,
            "pooledT": pooledT_np,
            "scores_f": scores_f_np,
            "A_rhs": A_np,
            "B_cat": B_np,
            "b_row": b_row_np,
            "msel": msel_np,
        })

    from concourse.bass_utils import run_bass_kernel_spmd
    res = run_bass_kernel_spmd(nc, in_maps, core_ids=list(range(N_CORES)))
    return np.stack([res.results[c]["out"] for c in range(N_CORES)], axis=0)


if __name__ == "__main__":
    rng = np.random.default_rng(0)
    demo = {
        "x": (rng.standard_normal((B, L, D)) * 0.02).astype(np.float32),
        "expert_scores": rng.random((B, E), dtype=np.float32),
        "W_base": (rng.standard_normal((D, D)) * 0.02).astype(np.float32),
        "b_base": np.zeros(D, np.float32),
        "gating_W": (rng.standard_normal((D, D)) * 0.02).astype(np.float32),
        "W_r": (rng.standard_normal((NMOD, D)) * 0.02).astype(np.float32),
        "lora_A": (rng.standard_normal((E, R, D)) * 0.02).astype(np.float32),
        "lora_B": (rng.standard_normal((E, D, R)) * 0.02).astype(np.float32),
        "module_idx": 0,
        "k": 2,
    }
    y = kernel(**demo)
    print("out", y.shape, y.dtype, float(np.abs(y).max()))
